# revision 2
# baseline (speedup 1.0000x reference)
"""Trainium2 Bass kernel for nn_MHLA_82695300317575.

Multi-head linear (relu-kernel) attention over 16-token windows with a
fixed 16x16 piece-mixing matrix, LayerNorm in, output projection out.

Strategy: pure data parallel over the batch dim (16 batches per core x 8
cores). Per (b,h) the window attention is computed in quadratic form:
  S^T = K Q^T  (256x256), scaled elementwise by kron(P, ones(16,16))^T,
  out^T = V^T S^T
which is mathematically identical to mixing the per-window kv matrices
with P and costs only large matmuls. All matmuls run as float32r
(full-rate fp32 PE mode). LayerNorm gamma is folded into the qkv weights
on the host; beta contributes bias terms handled on-device.
"""

import os
import sys

sys.path.insert(0, "/opt/trn_rl_repo")

import numpy as np

# Problem constants (hardcoded per harness contract)
B, N, W, C = 128, 16, 16, 768
H, D = 12, 64
NCORES = 8
BL = B // NCORES            # batches per core
TOK = BL * N * W            # tokens per core = 4096
TN = N * W                  # tokens per batch chunk = 256
EPS_ATTN = 1e-6
LN_EPS = 1e-5

LAST_EXEC_NS = None

_CACHE = {}


def _build_program():
    import concourse.tile as tile
    from concourse import bacc, mybir

    f32 = mybir.dt.float32
    f32r = mybir.dt.float32r
    AL = mybir.AluOpType
    AF = mybir.ActivationFunctionType

    nc = bacc.Bacc("TRN2", target_bir_lowering=False, debug=False,
                   num_devices=NCORES)

    x_d = nc.dram_tensor("x", [TOK, C], f32, kind="ExternalInput")
    wqk_d = nc.dram_tensor("wqk", [C, 2 * C], f32, kind="ExternalInput")
    wv_d = nc.dram_tensor("wv", [C, C], f32, kind="ExternalInput")
    wo_d = nc.dram_tensor("wo", [C, C], f32, kind="ExternalInput")
    bqk_d = nc.dram_tensor("bqk", [128, 12], f32, kind="ExternalInput")
    bvbc_d = nc.dram_tensor("bvbc", [128, C], f32, kind="ExternalInput")
    bobc_d = nc.dram_tensor("bobc", [128, C], f32, kind="ExternalInput")
    pbc_d = nc.dram_tensor("pbc", [128, 2, TN], f32, kind="ExternalInput")
    ident_d = nc.dram_tensor("ident", [128, 128], f32, kind="ExternalInput")
    y_d = nc.dram_tensor("y", [TOK, C], f32, kind="ExternalOutput")

    KT = C // 128            # 6 k-tiles over channels

    with tile.TileContext(nc) as tc:
        with (
            tc.tile_pool(name="singles", bufs=1) as singles,
            tc.tile_pool(name="io", bufs=2) as io,
            tc.tile_pool(name="pa", bufs=2) as pa,
            tc.tile_pool(name="pb", bufs=1) as pb,
            tc.tile_pool(name="sc", bufs=3) as sc,
            tc.tile_pool(name="st", bufs=4) as st,
            tc.tile_pool(name="ps", bufs=7, space="PSUM") as ps,
        ):
            wqk_sb = singles.tile([128, KT, 2 * C], f32r)
            nc.gpsimd.dma_start(wqk_sb[:], wqk_d.ap().rearrange(
                "(kt p) m -> p kt m", p=128))
            wv_sb = singles.tile([128, KT, C], f32r)
            nc.gpsimd.dma_start(wv_sb[:], wv_d.ap().rearrange(
                "(kt p) m -> p kt m", p=128))
            wo_sb = singles.tile([128, KT, C], f32r)
            nc.gpsimd.dma_start(wo_sb[:], wo_d.ap().rearrange(
                "(kt p) m -> p kt m", p=128))
            bqk_sb = singles.tile([128, 12], f32)
            nc.sync.dma_start(bqk_sb[:], bqk_d.ap())
            bvbc_sb = singles.tile([128, C], f32)
            nc.sync.dma_start(bvbc_sb[:], bvbc_d.ap())
            bobc_sb = singles.tile([128, C], f32)
            nc.sync.dma_start(bobc_sb[:], bobc_d.ap())
            pbc_sb = singles.tile([128, 2, TN], f32)
            nc.sync.dma_start(pbc_sb[:], pbc_d.ap())
            ident = singles.tile([128, 128], f32)
            nc.sync.dma_start(ident[:], ident_d.ap())
            eps_sb = singles.tile([128, 1], f32)
            nc.vector.memset(eps_sb[:], LN_EPS)

            for chunk in range(BL):
                t0 = chunk * TN

                # ---- load + LayerNorm (token-major, in place) ----
                xc = io.tile([128, 2, C], f32, tag="xc")
                nc.sync.dma_start(
                    xc[:], x_d.ap()[t0:t0 + TN].rearrange(
                        "(i p) c -> p i c", p=128))
                for ti in range(2):
                    stats = st.tile([128, 3, 6], f32, tag="bnst")
                    for s in range(3):
                        nc.vector.bn_stats(
                            stats[:, s, :], xc[:, ti, 256 * s:256 * (s + 1)])
                    mv = st.tile([128, 2], f32, tag="mv")
                    nc.vector.bn_aggr(mv[:], stats[:])
                    sq = st.tile([128, 1], f32, tag="sq")
                    nc.scalar.activation(sq[:], mv[:, 1:2], AF.Sqrt,
                                         bias=eps_sb[:], scale=1.0)
                    rstd = st.tile([128, 1], f32, tag="rstd")
                    nc.vector.reciprocal(rstd[:], sq[:])
                    negmu = st.tile([128, 1], f32, tag="ngm")
                    nc.vector.tensor_scalar(
                        negmu[:], mv[:, 0:1], scalar1=rstd[:], scalar2=-1.0,
                        op0=AL.mult, op1=AL.mult)
                    nc.vector.tensor_scalar(
                        xc[:, ti, :], xc[:, ti, :], scalar1=rstd[:],
                        scalar2=negmu[:], op0=AL.mult, op1=AL.add)

                # ---- transpose xn to channel-major [C, TN] ----
                xnT = pa.tile([128, KT, TN], f32r, tag="xnT")
                for ci in range(KT):
                    for ti in range(2):
                        pt = ps.tile([128, 512], f32, tag="ps")
                        nc.tensor.transpose(
                            pt[:, :128], xc[:, ti, 128 * ci:128 * (ci + 1)],
                            ident[:])
                        nc.scalar.copy(
                            xnT[:, ci, 128 * ti:128 * (ti + 1)], pt[:, :128])

                # ---- qk^T = Wqk^T @ xn^T, fused bias+relu+eps ----
                qkT = pb.tile([128, 12, TN], f32r, tag="qkT")
                for mi in range(12):
                    pq = ps.tile([128, 512], f32, tag="ps")
                    for kt in range(KT):
                        nc.tensor.matmul(
                            pq[:, :TN],
                            wqk_sb[:, kt, 128 * mi:128 * (mi + 1)],
                            xnT[:, kt, :],
                            start=(kt == 0), stop=(kt == KT - 1))
                    nc.vector.tensor_scalar(
                        qkT[:, mi, :], pq[:, :TN],
                        scalar1=bqk_sb[:, mi:mi + 1], scalar2=EPS_ATTN,
                        op0=AL.add, op1=AL.max)

                # ---- v = xn @ Wv (token-major) ----
                v_sb = pb.tile([128, 2, C], f32r, tag="v")
                for ti in range(2):
                    for n0, nn in ((0, 512), (512, 256)):
                        pv = ps.tile([128, 512], f32, tag="ps")
                        for kt in range(KT):
                            nc.tensor.matmul(
                                pv[:, :nn],
                                xnT[:, kt, 128 * ti:128 * (ti + 1)],
                                wv_sb[:, kt, n0:n0 + nn],
                                start=(kt == 0), stop=(kt == KT - 1))
                        nc.vector.tensor_tensor(
                            v_sb[:, ti, n0:n0 + nn], pv[:, :nn],
                            bvbc_sb[:, n0:n0 + nn], op=AL.add)

                # ---- attention per head: S^T = K Q^T (scaled), o^T = V^T S^T
                oT = pa.tile([128, KT, TN], f32r, tag="oT")
                for h in range(12):
                    ro = (h % 2) * 64
                    s_sb = sc.tile([128, 2, TN], f32r, tag="s")
                    for half in range(2):
                        pst = ps.tile([128, 512], f32, tag="ps")
                        nc.tensor.matmul(
                            pst[:, :TN],
                            qkT[ro:ro + 64, 6 + h // 2,
                                128 * half:128 * (half + 1)],
                            qkT[ro:ro + 64, h // 2, :],
                            start=True, stop=True)
                        nc.vector.tensor_tensor(
                            s_sb[:, half, :], pst[:, :TN],
                            pbc_sb[:, half, :], op=AL.mult)
                    po = ps.tile([128, 512], f32, tag="ps")
                    for kt in range(2):
                        nc.tensor.matmul(
                            po[:64, :TN],
                            v_sb[:, kt, 64 * h:64 * (h + 1)],
                            s_sb[:, kt, :],
                            start=(kt == 0), stop=(kt == 1))
                    nc.scalar.copy(oT[ro:ro + 64, h // 2, :], po[:64, :TN])

                # ---- y = out @ Wo + b_out (token-major) ----
                y_sb = io.tile([128, 2, C], f32, tag="y")
                for ti in range(2):
                    for n0, nn in ((0, 512), (512, 256)):
                        py = ps.tile([128, 512], f32, tag="ps")
                        for kt in range(KT):
                            nc.tensor.matmul(
                                py[:, :nn],
                                oT[:, kt, 128 * ti:128 * (ti + 1)],
                                wo_sb[:, kt, n0:n0 + nn],
                                start=(kt == 0), stop=(kt == KT - 1))
                        nc.vector.tensor_tensor(
                            y_sb[:, ti, n0:n0 + nn], py[:, :nn],
                            bobc_sb[:, n0:n0 + nn], op=AL.add)
                nc.sync.dma_start(
                    y_d.ap()[t0:t0 + TN].rearrange("(i p) c -> p i c", p=128),
                    y_sb[:])

    nc.compile()
    return nc


def _get_program():
    if "nc" not in _CACHE:
        _CACHE["nc"] = _build_program()
    return _CACHE["nc"]


def kernel(x, ln_gamma, ln_beta, w_qkv, w_out, b_out, w_piece):
    global LAST_EXEC_NS
    from concourse.bass_utils import run_bass_kernel_spmd

    x = np.asarray(x, dtype=np.float32)
    ln_gamma = np.asarray(ln_gamma, dtype=np.float32)
    ln_beta = np.asarray(ln_beta, dtype=np.float32)
    w_qkv = np.asarray(w_qkv, dtype=np.float32)
    w_out = np.asarray(w_out, dtype=np.float32)
    b_out = np.asarray(b_out, dtype=np.float32)
    w_piece = np.asarray(w_piece, dtype=np.float32)

    # Host-side weight prep: fold gamma into qkv weights; beta becomes biases.
    wqk = np.ascontiguousarray(ln_gamma[:, None] * w_qkv[:, :2 * C])
    wv = np.ascontiguousarray(ln_gamma[:, None] * w_qkv[:, 2 * C:])
    bqk = ln_beta @ w_qkv[:, :2 * C]
    bv = ln_beta @ w_qkv[:, 2 * C:]
    bqk_r = np.ascontiguousarray((bqk + EPS_ATTN).reshape(12, 128).T)
    bvbc = np.ascontiguousarray(np.broadcast_to(bv, (128, C)))
    bobc = np.ascontiguousarray(np.broadcast_to(b_out, (128, C)))
    # P^T broadcast tiles: pbc[p, half, f] = P[f//16, p//16 + 8*half]
    pk = np.kron(w_piece.T, np.ones((16, 16), dtype=np.float32))  # [256,256]
    pbc = np.ascontiguousarray(
        pk.reshape(2, 128, TN).transpose(1, 0, 2)).astype(np.float32)
    ident = np.eye(128, dtype=np.float32)
    wo = np.ascontiguousarray(w_out)

    shared = {
        "wqk": wqk, "wv": wv, "wo": wo, "bqk": bqk_r, "bvbc": bvbc,
        "bobc": bobc, "pbc": pbc, "ident": ident,
    }
    xs = x.reshape(NCORES, TOK, C)
    in_maps = [dict(shared, x=np.ascontiguousarray(xs[i]))
               for i in range(NCORES)]

    nc = _get_program()
    trace = bool(os.environ.get("MHLA_TRACE"))
    res = run_bass_kernel_spmd(nc, in_maps, core_ids=list(range(NCORES)),
                               trace=trace)
    LAST_EXEC_NS = res.exec_time_ns

    y = np.empty((NCORES, TOK, C), dtype=np.float32)
    for i in range(NCORES):
        y[i] = res.results[i]["y"]
    return y.reshape(B, N, W, C)


# revision 5
# speedup vs baseline: 1.1920x; 1.1920x over previous
"""Trainium2 Bass kernel for nn_MHLA_82695300317575.

Multi-head linear (relu-kernel) attention over 16-token windows with a
fixed 16x16 piece-mixing matrix, LayerNorm in, output projection out.

Strategy: pure data parallel over the batch dim (16 batches per core x 8
cores). Per (b,h) the window attention is computed in quadratic form:
  S^T = K Q^T  (256x256), scaled elementwise by kron(P, ones(16,16))^T,
  out^T = V^T S^T
which is mathematically identical to mixing the per-window kv matrices
with P and costs only large matmuls. All matmuls run as float32r
(full-rate fp32 PE mode); contraction dims are kept at the full 128
partitions (K<128 fp32r matmuls are ~4x slower), which is why q lives in
a zero-padded per-head tile. LayerNorm gamma is folded into the qkv
weights on the host; beta/b_out biases get dedicated program variants so
the common all-zero case skips the bias adds.
"""

import os
import sys

sys.path.insert(0, "/opt/trn_rl_repo")

import numpy as np

# Problem constants (hardcoded per harness contract)
B, N, W, C = 128, 16, 16, 768
H, D = 12, 64
NCORES = 8
BL = B // NCORES            # batches per core
TOK = BL * N * W            # tokens per core = 4096
TN = N * W                  # tokens per batch chunk = 256
EPS_ATTN = 1e-6
LN_EPS = 1e-5

LAST_EXEC_NS = None
LAST_RESULTS = None

_CACHE = {}


def _build_program(qk_bias, v_bias, o_bias):
    import concourse.tile as tile
    from concourse import bacc, mybir

    f32 = mybir.dt.float32
    f32r = mybir.dt.float32r
    AL = mybir.AluOpType
    AF = mybir.ActivationFunctionType

    nc = bacc.Bacc("TRN2", target_bir_lowering=False, debug=False,
                   num_devices=NCORES)

    x_d = nc.dram_tensor("x", [TOK, C], f32, kind="ExternalInput")
    wqk_d = nc.dram_tensor("wqk", [C, 2 * C], f32r, kind="ExternalInput")
    wv_d = nc.dram_tensor("wv", [C, C], f32r, kind="ExternalInput")
    wo_d = nc.dram_tensor("wo", [C, C], f32r, kind="ExternalInput")
    bqk_d = nc.dram_tensor("bqk", [128, 12], f32, kind="ExternalInput")
    bvbc_d = nc.dram_tensor("bvbc", [128, C], f32, kind="ExternalInput")
    bobc_d = nc.dram_tensor("bobc", [128, C], f32, kind="ExternalInput")
    pbc_d = nc.dram_tensor("pbc", [128, 2, TN], f32, kind="ExternalInput")
    ident_d = nc.dram_tensor("ident", [128, 128], f32, kind="ExternalInput")
    qz_d = nc.dram_tensor("qzero", [128, 12 * TN], f32r, kind="ExternalInput")
    y_d = nc.dram_tensor("y", [TOK, C], f32, kind="ExternalOutput")

    KT = C // 128            # 6 k-tiles over channels

    with tile.TileContext(nc) as tc:
        with (
            tc.tile_pool(name="singles", bufs=1) as singles,
            tc.tile_pool(name="io", bufs=2) as io,
            tc.tile_pool(name="pa", bufs=2) as pa,
            tc.tile_pool(name="pb", bufs=1) as pb,
            tc.tile_pool(name="sc", bufs=3) as sc,
            tc.tile_pool(name="st", bufs=4) as st,
            tc.tile_pool(name="ps_t", bufs=2, space="PSUM") as ps_t,
            tc.tile_pool(name="ps_mm", bufs=3, space="PSUM") as ps_mm,
            tc.tile_pool(name="ps_att", bufs=3, space="PSUM") as ps_att,
        ):
            wqk_sb = singles.tile([128, KT, 2 * C], f32r)
            nc.sync.dma_start(wqk_sb[:], wqk_d.ap().rearrange(
                "(kt p) m -> p kt m", p=128))
            wv_sb = singles.tile([128, KT, C], f32r)
            nc.sync.dma_start(wv_sb[:], wv_d.ap().rearrange(
                "(kt p) m -> p kt m", p=128))
            wo_sb = singles.tile([128, KT, C], f32r)
            nc.sync.dma_start(wo_sb[:], wo_d.ap().rearrange(
                "(kt p) m -> p kt m", p=128))
            if qk_bias:
                bqk_sb = singles.tile([128, 12], f32)
                nc.sync.dma_start(bqk_sb[:], bqk_d.ap())
            if v_bias:
                bvbc_sb = singles.tile([128, C], f32)
                nc.sync.dma_start(bvbc_sb[:], bvbc_d.ap())
            if o_bias:
                bobc_sb = singles.tile([128, C], f32)
                nc.sync.dma_start(bobc_sb[:], bobc_d.ap())
            pbc_sb = singles.tile([128, 2, TN], f32)
            nc.sync.dma_start(pbc_sb[:], pbc_d.ap())
            ident = singles.tile([128, 128], f32)
            nc.sync.dma_start(ident[:], ident_d.ap())
            eps_sb = singles.tile([128, 1], f32)
            nc.vector.memset(eps_sb[:], LN_EPS)

            # Persistent zero-padded q tiles (double buffered by chunk
            # parity). Per head h, rows (h%2)*64..+64 hold relu(q_h)+eps;
            # the other 64 rows stay zero so the scores matmul can run with
            # the full K=128 contraction against a k tile whose complementary
            # rows hold the sibling head (junk x 0 = 0).
            qpads = []
            for i in range(2):
                qp = singles.tile([128, 12, TN], f32r, tag=f"qpad{i}")
                nc.sync.dma_start(
                    qp[:], qz_d.ap().rearrange("p (h t) -> p h t", h=12))
                qpads.append(qp)

            for chunk in range(BL):
                t0 = chunk * TN
                qpad = qpads[chunk % 2]

                # ---- load + LayerNorm (token-major, in place) ----
                xc = io.tile([128, 2, C], f32, tag="xc")
                nc.sync.dma_start(
                    xc[:], x_d.ap()[t0:t0 + TN].rearrange(
                        "(i p) c -> p i c", p=128))
                for ti in range(2):
                    stats = st.tile([128, 3, 6], f32, tag="bnst")
                    for s in range(3):
                        nc.vector.bn_stats(
                            stats[:, s, :], xc[:, ti, 256 * s:256 * (s + 1)])
                    mv = st.tile([128, 2], f32, tag="mv")
                    nc.vector.bn_aggr(mv[:], stats[:])
                    sq = st.tile([128, 1], f32, tag="sq")
                    nc.scalar.activation(sq[:], mv[:, 1:2], AF.Sqrt,
                                         bias=eps_sb[:], scale=1.0)
                    rstd = st.tile([128, 1], f32, tag="rstd")
                    nc.vector.reciprocal(rstd[:], sq[:])
                    negmu = st.tile([128, 1], f32, tag="ngm")
                    nc.vector.tensor_scalar(
                        negmu[:], mv[:, 0:1], scalar1=rstd[:], scalar2=-1.0,
                        op0=AL.mult, op1=AL.mult)
                    nc.gpsimd.tensor_scalar(
                        xc[:, ti, :], xc[:, ti, :], scalar1=rstd[:],
                        scalar2=negmu[:], op0=AL.mult, op1=AL.add)

                # ---- transpose xn to channel-major [C, TN] ----
                xnT = pa.tile([128, KT, TN], f32r, tag="xnT")
                for ci in range(KT):
                    for ti in range(2):
                        pt = ps_t.tile([128, 128], f32, tag="ps")
                        nc.tensor.transpose(
                            pt[:], xc[:, ti, 128 * ci:128 * (ci + 1)],
                            ident[:])
                        nc.scalar.copy(
                            xnT[:, ci, 128 * ti:128 * (ti + 1)], pt[:])

                # ---- qk^T = Wqk^T @ xn^T, fused bias+relu+eps ----
                # mi 0..5 produce q (written per-head into qpad);
                # mi 6..11 produce k (written into kT).
                kT = pb.tile([128, KT, TN], f32r, tag="kT")
                for mi in range(12):
                    pq = ps_mm.tile([128, 512], f32, tag="ps")
                    for kt in range(KT):
                        nc.tensor.matmul(
                            pq[:, :TN],
                            wqk_sb[:, kt, 128 * mi:128 * (mi + 1)],
                            xnT[:, kt, :],
                            start=(kt == 0), stop=(kt == KT - 1))
                    if mi < 6:
                        for par in range(2):
                            h = 2 * mi + par
                            ro = par * 64
                            if qk_bias:
                                nc.vector.tensor_scalar(
                                    qpad[ro:ro + 64, h, :],
                                    pq[ro:ro + 64, :TN],
                                    scalar1=bqk_sb[ro:ro + 64, mi:mi + 1],
                                    scalar2=EPS_ATTN, op0=AL.add, op1=AL.max)
                            else:
                                nc.vector.tensor_scalar(
                                    qpad[ro:ro + 64, h, :],
                                    pq[ro:ro + 64, :TN],
                                    scalar1=EPS_ATTN, scalar2=EPS_ATTN,
                                    op0=AL.add, op1=AL.max)
                    else:
                        if qk_bias:
                            nc.vector.tensor_scalar(
                                kT[:, mi - 6, :], pq[:, :TN],
                                scalar1=bqk_sb[:, mi:mi + 1],
                                scalar2=EPS_ATTN, op0=AL.add, op1=AL.max)
                        else:
                            nc.vector.tensor_scalar(
                                kT[:, mi - 6, :], pq[:, :TN],
                                scalar1=EPS_ATTN, scalar2=EPS_ATTN,
                                op0=AL.add, op1=AL.max)

                # ---- v = xn @ Wv (token-major) ----
                v_sb = pb.tile([128, 2, C], f32r, tag="v")
                for ti in range(2):
                    for n0, nn in ((0, 512), (512, 256)):
                        pv = ps_mm.tile([128, 512], f32, tag="ps")
                        for kt in range(KT):
                            nc.tensor.matmul(
                                pv[:, :nn],
                                xnT[:, kt, 128 * ti:128 * (ti + 1)],
                                wv_sb[:, kt, n0:n0 + nn],
                                start=(kt == 0), stop=(kt == KT - 1))
                        if v_bias:
                            nc.vector.tensor_tensor(
                                v_sb[:, ti, n0:n0 + nn], pv[:, :nn],
                                bvbc_sb[:, n0:n0 + nn], op=AL.add)
                        else:
                            nc.scalar.copy(
                                v_sb[:, ti, n0:n0 + nn], pv[:, :nn])

                # ---- attention per head: S^T = K Q^T (scaled), o^T = V^T S^T
                oT = pa.tile([128, KT, TN], f32r, tag="oT")
                for h in range(12):
                    ro = (h % 2) * 64
                    s_sb = sc.tile([128, 2, TN], f32r, tag="s")
                    for half in range(2):
                        pst = ps_att.tile([128, 256], f32, tag="ps")
                        nc.tensor.matmul(
                            pst[:],
                            kT[:, h // 2, 128 * half:128 * (half + 1)],
                            qpad[:, h, :],
                            start=True, stop=True)
                        nc.vector.tensor_tensor(
                            s_sb[:, half, :], pst[:],
                            pbc_sb[:, half, :], op=AL.mult)
                    po = ps_att.tile([128, 256], f32, tag="ps")
                    for kt in range(2):
                        nc.tensor.matmul(
                            po[:64, :],
                            v_sb[:, kt, 64 * h:64 * (h + 1)],
                            s_sb[:, kt, :],
                            start=(kt == 0), stop=(kt == 1))
                    nc.scalar.copy(oT[ro:ro + 64, h // 2, :], po[:64, :])

                # ---- y = out @ Wo + b_out (token-major) ----
                y_sb = io.tile([128, 2, C], f32, tag="y")
                for ti in range(2):
                    for n0, nn in ((0, 512), (512, 256)):
                        py = ps_mm.tile([128, 512], f32, tag="ps")
                        for kt in range(KT):
                            nc.tensor.matmul(
                                py[:, :nn],
                                oT[:, kt, 128 * ti:128 * (ti + 1)],
                                wo_sb[:, kt, n0:n0 + nn],
                                start=(kt == 0), stop=(kt == KT - 1))
                        if o_bias:
                            nc.vector.tensor_tensor(
                                y_sb[:, ti, n0:n0 + nn], py[:, :nn],
                                bobc_sb[:, n0:n0 + nn], op=AL.add)
                        else:
                            nc.scalar.copy(
                                y_sb[:, ti, n0:n0 + nn], py[:, :nn])
                nc.sync.dma_start(
                    y_d.ap()[t0:t0 + TN].rearrange("(i p) c -> p i c", p=128),
                    y_sb[:])

    nc.compile()
    return nc


def _get_program(qk_bias, v_bias, o_bias):
    key = (qk_bias, v_bias, o_bias)
    if key not in _CACHE:
        _CACHE[key] = _build_program(*key)
    return _CACHE[key]


def _round_f32r(a):
    """Round to the bf16-pair representable set (what fp32r matmuls use)."""
    import ml_dtypes
    hi = a.astype(ml_dtypes.bfloat16).astype(np.float32)
    lo = (a - hi).astype(ml_dtypes.bfloat16).astype(np.float32)
    return hi + lo


def kernel(x, ln_gamma, ln_beta, w_qkv, w_out, b_out, w_piece):
    global LAST_EXEC_NS, LAST_RESULTS
    from concourse.bass_utils import run_bass_kernel_spmd

    x = np.asarray(x, dtype=np.float32)
    ln_gamma = np.asarray(ln_gamma, dtype=np.float32)
    ln_beta = np.asarray(ln_beta, dtype=np.float32)
    w_qkv = np.asarray(w_qkv, dtype=np.float32)
    w_out = np.asarray(w_out, dtype=np.float32)
    b_out = np.asarray(b_out, dtype=np.float32)
    w_piece = np.asarray(w_piece, dtype=np.float32)

    # Host-side weight prep: fold gamma into qkv weights; beta becomes biases.
    wqk = _round_f32r(np.ascontiguousarray(ln_gamma[:, None] * w_qkv[:, :2 * C]))
    wv = _round_f32r(np.ascontiguousarray(ln_gamma[:, None] * w_qkv[:, 2 * C:]))
    wo = _round_f32r(np.ascontiguousarray(w_out))
    bqk = ln_beta @ w_qkv[:, :2 * C]
    bv = ln_beta @ w_qkv[:, 2 * C:]
    qk_bias = bool(np.any(bqk))
    v_bias = bool(np.any(bv))
    o_bias = bool(np.any(b_out))
    bqk_r = np.ascontiguousarray((bqk + EPS_ATTN).reshape(12, 128).T)
    bvbc = np.ascontiguousarray(np.broadcast_to(bv, (128, C)))
    bobc = np.ascontiguousarray(np.broadcast_to(b_out, (128, C)))
    # P^T broadcast tiles: pbc[p, half, f] = P[f//16, p//16 + 8*half]
    pk = np.kron(w_piece.T, np.ones((16, 16), dtype=np.float32))  # [256,256]
    pbc = np.ascontiguousarray(
        pk.reshape(2, 128, TN).transpose(1, 0, 2)).astype(np.float32)
    ident = np.eye(128, dtype=np.float32)

    shared = {
        "wqk": wqk, "wv": wv, "wo": wo, "bqk": bqk_r, "bvbc": bvbc,
        "bobc": bobc, "pbc": pbc, "ident": ident,
        "qzero": np.zeros((128, 12 * TN), dtype=np.float32),
    }
    xs = x.reshape(NCORES, TOK, C)
    in_maps = [dict(shared, x=np.ascontiguousarray(xs[i]))
               for i in range(NCORES)]

    nc = _get_program(qk_bias, v_bias, o_bias)
    trace = bool(os.environ.get("MHLA_TRACE"))
    res = run_bass_kernel_spmd(nc, in_maps, core_ids=list(range(NCORES)),
                               trace=trace)
    LAST_EXEC_NS = res.exec_time_ns
    LAST_RESULTS = res

    y = np.empty((NCORES, TOK, C), dtype=np.float32)
    for i in range(NCORES):
        y[i] = res.results[i]["y"]
    return y.reshape(B, N, W, C)


# revision 6
# speedup vs baseline: 1.2155x; 1.0197x over previous
"""Trainium2 Bass kernel for nn_MHLA_82695300317575.

Multi-head linear (relu-kernel) attention over 16-token windows with a
fixed 16x16 piece-mixing matrix, LayerNorm in, output projection out.

Strategy: pure data parallel over the batch dim (16 batches per core x 8
cores). Per (b,h) the window attention is computed in quadratic form:
  S^T = K Q^T  (256x256), scaled elementwise by kron(P, ones(16,16))^T,
  out^T = V^T S^T
which is mathematically identical to mixing the per-window kv matrices
with P and costs only large matmuls. All matmuls run as float32r
(full-rate fp32 PE mode); contraction dims are kept at the full 128
partitions (K<128 fp32r matmuls are ~4x slower), which is why q lives in
a zero-padded per-head tile. LayerNorm gamma is folded into the qkv
weights on the host; beta/b_out biases get dedicated program variants so
the common all-zero case skips the bias adds.
"""

import os
import sys

sys.path.insert(0, "/opt/trn_rl_repo")

import numpy as np

# Problem constants (hardcoded per harness contract)
B, N, W, C = 128, 16, 16, 768
H, D = 12, 64
NCORES = 8
BL = B // NCORES            # batches per core
TOK = BL * N * W            # tokens per core = 4096
TN = N * W                  # tokens per batch chunk = 256
EPS_ATTN = 1e-6
LN_EPS = 1e-5

LAST_EXEC_NS = None
LAST_RESULTS = None

_CACHE = {}


def _build_program(qk_bias, v_bias, o_bias):
    import concourse.tile as tile
    from concourse import bacc, mybir

    f32 = mybir.dt.float32
    f32r = mybir.dt.float32r
    AL = mybir.AluOpType
    AF = mybir.ActivationFunctionType

    nc = bacc.Bacc("TRN2", target_bir_lowering=False, debug=False,
                   num_devices=NCORES)

    x_d = nc.dram_tensor("x", [TOK, C], f32, kind="ExternalInput")
    wqk_d = nc.dram_tensor("wqk", [C, 2 * C], f32r, kind="ExternalInput")
    wv_d = nc.dram_tensor("wv", [C, C], f32r, kind="ExternalInput")
    wo_d = nc.dram_tensor("wo", [C, C], f32r, kind="ExternalInput")
    bqk_d = nc.dram_tensor("bqk", [128, 12], f32, kind="ExternalInput")
    bvbc_d = nc.dram_tensor("bvbc", [128, C], f32, kind="ExternalInput")
    bobc_d = nc.dram_tensor("bobc", [128, C], f32, kind="ExternalInput")
    pbc_d = nc.dram_tensor("pbc", [128, 2, TN], f32, kind="ExternalInput")
    ident_d = nc.dram_tensor("ident", [128, 128], f32, kind="ExternalInput")
    qz_d = nc.dram_tensor("qzero", [128, 12 * TN], f32r, kind="ExternalInput")
    y_d = nc.dram_tensor("y", [TOK, C], f32, kind="ExternalOutput")

    KT = C // 128            # 6 k-tiles over channels

    with tile.TileContext(nc) as tc:
        with (
            tc.tile_pool(name="singles", bufs=1) as singles,
            tc.tile_pool(name="io", bufs=2) as io,
            tc.tile_pool(name="pa", bufs=2) as pa,
            tc.tile_pool(name="pb", bufs=1) as pb,
            tc.tile_pool(name="sc", bufs=5) as sc,
            tc.tile_pool(name="st", bufs=4) as st,
            tc.tile_pool(name="ps_t", bufs=2, space="PSUM") as ps_t,
            tc.tile_pool(name="ps_mm", bufs=3, space="PSUM") as ps_mm,
            tc.tile_pool(name="ps_att", bufs=3, space="PSUM") as ps_att,
        ):
            wqk_sb = singles.tile([128, KT, 2 * C], f32r)
            nc.sync.dma_start(wqk_sb[:], wqk_d.ap().rearrange(
                "(kt p) m -> p kt m", p=128))
            wv_sb = singles.tile([128, KT, C], f32r)
            nc.sync.dma_start(wv_sb[:], wv_d.ap().rearrange(
                "(kt p) m -> p kt m", p=128))
            wo_sb = singles.tile([128, KT, C], f32r)
            nc.sync.dma_start(wo_sb[:], wo_d.ap().rearrange(
                "(kt p) m -> p kt m", p=128))
            if qk_bias:
                bqk_sb = singles.tile([128, 12], f32)
                nc.sync.dma_start(bqk_sb[:], bqk_d.ap())
            if v_bias:
                bvbc_sb = singles.tile([128, C], f32)
                nc.sync.dma_start(bvbc_sb[:], bvbc_d.ap())
            if o_bias:
                bobc_sb = singles.tile([128, C], f32)
                nc.sync.dma_start(bobc_sb[:], bobc_d.ap())
            pbc_sb = singles.tile([128, 2, TN], f32)
            nc.sync.dma_start(pbc_sb[:], pbc_d.ap())
            ident = singles.tile([128, 128], f32)
            nc.sync.dma_start(ident[:], ident_d.ap())
            eps_sb = singles.tile([128, 1], f32)
            nc.vector.memset(eps_sb[:], LN_EPS)

            # Persistent zero-padded q tiles (double buffered by chunk
            # parity). Per head h, rows (h%2)*64..+64 hold relu(q_h)+eps;
            # the other 64 rows stay zero so the scores matmul can run with
            # the full K=128 contraction against a k tile whose complementary
            # rows hold the sibling head (junk x 0 = 0).
            qpads = []
            for i in range(2):
                qp = singles.tile([128, 12, TN], f32r, tag=f"qpad{i}")
                nc.sync.dma_start(
                    qp[:], qz_d.ap().rearrange("p (h t) -> p h t", h=12))
                qpads.append(qp)

            for chunk in range(BL):
                t0 = chunk * TN
                qpad = qpads[chunk % 2]

                # ---- load + LayerNorm (token-major, in place) ----
                xc = io.tile([128, 2, C], f32, tag="xc")
                nc.sync.dma_start(
                    xc[:], x_d.ap()[t0:t0 + TN].rearrange(
                        "(i p) c -> p i c", p=128))
                for ti in range(2):
                    stats = st.tile([128, 3, 6], f32, tag="bnst")
                    for s in range(3):
                        nc.vector.bn_stats(
                            stats[:, s, :], xc[:, ti, 256 * s:256 * (s + 1)])
                    mv = st.tile([128, 2], f32, tag="mv")
                    nc.vector.bn_aggr(mv[:], stats[:])
                    sq = st.tile([128, 1], f32, tag="sq")
                    nc.scalar.activation(sq[:], mv[:, 1:2], AF.Sqrt,
                                         bias=eps_sb[:], scale=1.0)
                    rstd = st.tile([128, 1], f32, tag="rstd")
                    nc.vector.reciprocal(rstd[:], sq[:])
                    negmu = st.tile([128, 1], f32, tag="ngm")
                    nc.vector.tensor_scalar(
                        negmu[:], mv[:, 0:1], scalar1=rstd[:], scalar2=-1.0,
                        op0=AL.mult, op1=AL.mult)
                    nc.gpsimd.tensor_scalar(
                        xc[:, ti, :], xc[:, ti, :], scalar1=rstd[:],
                        scalar2=negmu[:], op0=AL.mult, op1=AL.add)

                # ---- transpose xn to channel-major [C, TN] ----
                xnT = pa.tile([128, KT, TN], f32r, tag="xnT")
                for ci in range(KT):
                    for ti in range(2):
                        pt = ps_t.tile([128, 128], f32, tag="ps")
                        nc.tensor.transpose(
                            pt[:], xc[:, ti, 128 * ci:128 * (ci + 1)],
                            ident[:])
                        nc.scalar.copy(
                            xnT[:, ci, 128 * ti:128 * (ti + 1)], pt[:])

                # ---- qk^T = Wqk^T @ xn^T, fused bias+relu+eps ----
                # mi 0..5 produce q (written per-head into qpad);
                # mi 6..11 produce k (written into kT).
                kT = pb.tile([128, KT, TN], f32r, tag="kT")
                for mi in range(12):
                    pq = ps_mm.tile([128, 512], f32, tag="ps")
                    for kt in range(KT):
                        nc.tensor.matmul(
                            pq[:, :TN],
                            wqk_sb[:, kt, 128 * mi:128 * (mi + 1)],
                            xnT[:, kt, :],
                            start=(kt == 0), stop=(kt == KT - 1))
                    if mi < 6:
                        for par in range(2):
                            h = 2 * mi + par
                            ro = par * 64
                            if qk_bias:
                                nc.vector.tensor_scalar(
                                    qpad[ro:ro + 64, h, :],
                                    pq[ro:ro + 64, :TN],
                                    scalar1=bqk_sb[ro:ro + 64, mi:mi + 1],
                                    scalar2=EPS_ATTN, op0=AL.add, op1=AL.max)
                            else:
                                nc.vector.tensor_scalar(
                                    qpad[ro:ro + 64, h, :],
                                    pq[ro:ro + 64, :TN],
                                    scalar1=EPS_ATTN, scalar2=EPS_ATTN,
                                    op0=AL.add, op1=AL.max)
                    else:
                        if qk_bias:
                            nc.vector.tensor_scalar(
                                kT[:, mi - 6, :], pq[:, :TN],
                                scalar1=bqk_sb[:, mi:mi + 1],
                                scalar2=EPS_ATTN, op0=AL.add, op1=AL.max)
                        else:
                            nc.vector.tensor_scalar(
                                kT[:, mi - 6, :], pq[:, :TN],
                                scalar1=EPS_ATTN, scalar2=EPS_ATTN,
                                op0=AL.add, op1=AL.max)

                # ---- v = xn @ Wv (token-major) ----
                v_sb = pb.tile([128, 2, C], f32r, tag="v")
                for ti in range(2):
                    for n0, nn in ((0, 512), (512, 256)):
                        pv = ps_mm.tile([128, 512], f32, tag="ps")
                        for kt in range(KT):
                            nc.tensor.matmul(
                                pv[:, :nn],
                                xnT[:, kt, 128 * ti:128 * (ti + 1)],
                                wv_sb[:, kt, n0:n0 + nn],
                                start=(kt == 0), stop=(kt == KT - 1))
                        if v_bias:
                            nc.vector.tensor_tensor(
                                v_sb[:, ti, n0:n0 + nn], pv[:, :nn],
                                bvbc_sb[:, n0:n0 + nn], op=AL.add)
                        else:
                            nc.scalar.copy(
                                v_sb[:, ti, n0:n0 + nn], pv[:, :nn])

                # ---- attention per head: S^T = K Q^T (scaled), o^T = V^T S^T
                # Software-pipelined: outT lags scores by LAG heads so PE is
                # not stalled on the elementwise piece-scale of the same head.
                oT = pa.tile([128, KT, TN], f32r, tag="oT")
                LAG = 3
                s_list = [None] * 12
                for step in range(12 + LAG):
                    if step < 12:
                        h = step
                        s_sb = sc.tile([128, 2, TN], f32r, tag="s")
                        s_list[h] = s_sb
                        for half in range(2):
                            pst = ps_att.tile([128, 256], f32, tag="ps")
                            nc.tensor.matmul(
                                pst[:],
                                kT[:, h // 2, 128 * half:128 * (half + 1)],
                                qpad[:, h, :],
                                start=True, stop=True)
                            if half == 0:
                                nc.vector.tensor_tensor(
                                    s_sb[:, half, :], pst[:],
                                    pbc_sb[:, half, :], op=AL.mult)
                            else:
                                nc.scalar.copy(s_sb[:, half, :], pst[:])
                                nc.gpsimd.tensor_tensor(
                                    s_sb[:, half, :], s_sb[:, half, :],
                                    pbc_sb[:, half, :], op=AL.mult)
                    if step >= LAG:
                        h = step - LAG
                        ro = (h % 2) * 64
                        s_sb = s_list[h]
                        po = ps_att.tile([128, 256], f32, tag="ps")
                        for kt in range(2):
                            nc.tensor.matmul(
                                po[:64, :],
                                v_sb[:, kt, 64 * h:64 * (h + 1)],
                                s_sb[:, kt, :],
                                start=(kt == 0), stop=(kt == 1))
                        nc.scalar.copy(oT[ro:ro + 64, h // 2, :], po[:64, :])

                # ---- y = out @ Wo + b_out (token-major) ----
                y_sb = io.tile([128, 2, C], f32, tag="y")
                for ti in range(2):
                    for n0, nn in ((0, 512), (512, 256)):
                        py = ps_mm.tile([128, 512], f32, tag="ps")
                        for kt in range(KT):
                            nc.tensor.matmul(
                                py[:, :nn],
                                oT[:, kt, 128 * ti:128 * (ti + 1)],
                                wo_sb[:, kt, n0:n0 + nn],
                                start=(kt == 0), stop=(kt == KT - 1))
                        if o_bias:
                            nc.vector.tensor_tensor(
                                y_sb[:, ti, n0:n0 + nn], py[:, :nn],
                                bobc_sb[:, n0:n0 + nn], op=AL.add)
                        else:
                            nc.scalar.copy(
                                y_sb[:, ti, n0:n0 + nn], py[:, :nn])
                nc.sync.dma_start(
                    y_d.ap()[t0:t0 + TN].rearrange("(i p) c -> p i c", p=128),
                    y_sb[:])

    nc.compile()
    return nc


def _get_program(qk_bias, v_bias, o_bias):
    key = (qk_bias, v_bias, o_bias)
    if key not in _CACHE:
        _CACHE[key] = _build_program(*key)
    return _CACHE[key]


def _round_f32r(a):
    """Round to the bf16-pair representable set (what fp32r matmuls use)."""
    import ml_dtypes
    hi = a.astype(ml_dtypes.bfloat16).astype(np.float32)
    lo = (a - hi).astype(ml_dtypes.bfloat16).astype(np.float32)
    return hi + lo


def kernel(x, ln_gamma, ln_beta, w_qkv, w_out, b_out, w_piece):
    global LAST_EXEC_NS, LAST_RESULTS
    from concourse.bass_utils import run_bass_kernel_spmd

    x = np.asarray(x, dtype=np.float32)
    ln_gamma = np.asarray(ln_gamma, dtype=np.float32)
    ln_beta = np.asarray(ln_beta, dtype=np.float32)
    w_qkv = np.asarray(w_qkv, dtype=np.float32)
    w_out = np.asarray(w_out, dtype=np.float32)
    b_out = np.asarray(b_out, dtype=np.float32)
    w_piece = np.asarray(w_piece, dtype=np.float32)

    # Host-side weight prep: fold gamma into qkv weights; beta becomes biases.
    wqk = _round_f32r(np.ascontiguousarray(ln_gamma[:, None] * w_qkv[:, :2 * C]))
    wv = _round_f32r(np.ascontiguousarray(ln_gamma[:, None] * w_qkv[:, 2 * C:]))
    wo = _round_f32r(np.ascontiguousarray(w_out))
    bqk = ln_beta @ w_qkv[:, :2 * C]
    bv = ln_beta @ w_qkv[:, 2 * C:]
    qk_bias = bool(np.any(bqk))
    v_bias = bool(np.any(bv))
    o_bias = bool(np.any(b_out))
    bqk_r = np.ascontiguousarray((bqk + EPS_ATTN).reshape(12, 128).T)
    bvbc = np.ascontiguousarray(np.broadcast_to(bv, (128, C)))
    bobc = np.ascontiguousarray(np.broadcast_to(b_out, (128, C)))
    # P^T broadcast tiles: pbc[p, half, f] = P[f//16, p//16 + 8*half]
    pk = np.kron(w_piece.T, np.ones((16, 16), dtype=np.float32))  # [256,256]
    pbc = np.ascontiguousarray(
        pk.reshape(2, 128, TN).transpose(1, 0, 2)).astype(np.float32)
    ident = np.eye(128, dtype=np.float32)

    shared = {
        "wqk": wqk, "wv": wv, "wo": wo, "bqk": bqk_r, "bvbc": bvbc,
        "bobc": bobc, "pbc": pbc, "ident": ident,
        "qzero": np.zeros((128, 12 * TN), dtype=np.float32),
    }
    xs = x.reshape(NCORES, TOK, C)
    in_maps = [dict(shared, x=np.ascontiguousarray(xs[i]))
               for i in range(NCORES)]

    nc = _get_program(qk_bias, v_bias, o_bias)
    trace = bool(os.environ.get("MHLA_TRACE"))
    res = run_bass_kernel_spmd(nc, in_maps, core_ids=list(range(NCORES)),
                               trace=trace)
    LAST_EXEC_NS = res.exec_time_ns
    LAST_RESULTS = res

    y = np.empty((NCORES, TOK, C), dtype=np.float32)
    for i in range(NCORES):
        y[i] = res.results[i]["y"]
    return y.reshape(B, N, W, C)


# revision 7
# speedup vs baseline: 1.2303x; 1.0122x over previous
"""Trainium2 Bass kernel for nn_MHLA_82695300317575.

Multi-head linear (relu-kernel) attention over 16-token windows with a
fixed 16x16 piece-mixing matrix, LayerNorm in, output projection out.

Strategy: pure data parallel over the batch dim (16 batches per core x 8
cores). Per (b,h) the window attention is computed in quadratic form:
  S^T = K Q^T  (256x256), scaled elementwise by kron(P, ones(16,16))^T,
  out^T = V^T S^T
which is mathematically identical to mixing the per-window kv matrices
with P and costs only large matmuls. All matmuls run as float32r
(full-rate fp32 PE mode); contraction dims are kept at the full 128
partitions (K<128 fp32r matmuls are ~4x slower), which is why q lives in
a zero-padded per-head tile. LayerNorm gamma is folded into the qkv
weights on the host; beta/b_out biases get dedicated program variants so
the common all-zero case skips the bias adds.
"""

import os
import sys

sys.path.insert(0, "/opt/trn_rl_repo")

import numpy as np

# Problem constants (hardcoded per harness contract)
B, N, W, C = 128, 16, 16, 768
H, D = 12, 64
NCORES = 8
BL = B // NCORES            # batches per core
TOK = BL * N * W            # tokens per core = 4096
TN = N * W                  # tokens per batch chunk = 256
EPS_ATTN = 1e-6
LN_EPS = 1e-5

LAST_EXEC_NS = None
LAST_RESULTS = None

_CACHE = {}


def _build_program(qk_bias, v_bias, o_bias):
    import concourse.tile as tile
    from concourse import bacc, mybir

    f32 = mybir.dt.float32
    f32r = mybir.dt.float32r
    AL = mybir.AluOpType
    AF = mybir.ActivationFunctionType

    nc = bacc.Bacc("TRN2", target_bir_lowering=False, debug=False,
                   num_devices=NCORES)

    x_d = nc.dram_tensor("x", [TOK, C], f32, kind="ExternalInput")
    wqk_d = nc.dram_tensor("wqk", [C, 2 * C], f32r, kind="ExternalInput")
    wv_d = nc.dram_tensor("wv", [C, C], f32r, kind="ExternalInput")
    wo_d = nc.dram_tensor("wo", [C, C], f32r, kind="ExternalInput")
    bqk_d = nc.dram_tensor("bqk", [128, 12], f32, kind="ExternalInput")
    bvbc_d = nc.dram_tensor("bvbc", [128, C], f32, kind="ExternalInput")
    bobc_d = nc.dram_tensor("bobc", [128, C], f32, kind="ExternalInput")
    pbc_d = nc.dram_tensor("pbc", [128, 2, TN], f32, kind="ExternalInput")
    ident_d = nc.dram_tensor("ident", [128, 128], f32, kind="ExternalInput")
    qz_d = nc.dram_tensor("qzero", [128, 12 * TN], f32r, kind="ExternalInput")
    y_d = nc.dram_tensor("y", [TOK, C], f32, kind="ExternalOutput")

    KT = C // 128            # 6 k-tiles over channels

    with tile.TileContext(nc) as tc:
        with (
            tc.tile_pool(name="singles", bufs=1) as singles,
            tc.tile_pool(name="io", bufs=2) as io,
            tc.tile_pool(name="pa", bufs=2) as pa,
            tc.tile_pool(name="pb", bufs=1) as pb,
            tc.tile_pool(name="sc", bufs=5) as sc,
            tc.tile_pool(name="st", bufs=4) as st,
            tc.tile_pool(name="ps_t", bufs=2, space="PSUM") as ps_t,
            tc.tile_pool(name="ps_mm", bufs=3, space="PSUM") as ps_mm,
            tc.tile_pool(name="ps_att", bufs=3, space="PSUM") as ps_att,
        ):
            wqk_sb = singles.tile([128, KT, 2 * C], f32r)
            nc.gpsimd.dma_start(wqk_sb[:], wqk_d.ap().rearrange(
                "(kt p) m -> p kt m", p=128))
            wv_sb = singles.tile([128, KT, C], f32r)
            nc.gpsimd.dma_start(wv_sb[:], wv_d.ap().rearrange(
                "(kt p) m -> p kt m", p=128))
            wo_sb = singles.tile([128, KT, C], f32r)
            nc.gpsimd.dma_start(wo_sb[:], wo_d.ap().rearrange(
                "(kt p) m -> p kt m", p=128))
            if qk_bias:
                bqk_sb = singles.tile([128, 12], f32)
                nc.gpsimd.dma_start(bqk_sb[:], bqk_d.ap())
            if v_bias:
                bvbc_sb = singles.tile([128, C], f32)
                nc.gpsimd.dma_start(bvbc_sb[:], bvbc_d.ap())
            if o_bias:
                bobc_sb = singles.tile([128, C], f32)
                nc.gpsimd.dma_start(bobc_sb[:], bobc_d.ap())
            pbc_sb = singles.tile([128, 2, TN], f32)
            nc.gpsimd.dma_start(pbc_sb[:], pbc_d.ap())
            ident = singles.tile([128, 128], f32)
            nc.sync.dma_start(ident[:], ident_d.ap())
            eps_sb = singles.tile([128, 1], f32)
            nc.vector.memset(eps_sb[:], LN_EPS)

            # Persistent zero-padded q tiles (double buffered by chunk
            # parity). Per head h, rows (h%2)*64..+64 hold relu(q_h)+eps;
            # the other 64 rows stay zero so the scores matmul can run with
            # the full K=128 contraction against a k tile whose complementary
            # rows hold the sibling head (junk x 0 = 0).
            qpads = []
            for i in range(2):
                qp = singles.tile([128, 12, TN], f32r, tag=f"qpad{i}")
                nc.gpsimd.dma_start(
                    qp[:], qz_d.ap().rearrange("p (h t) -> p h t", h=12))
                qpads.append(qp)

            for chunk in range(BL):
                t0 = chunk * TN
                qpad = qpads[chunk % 2]

                # ---- load + LayerNorm (token-major, in place) ----
                xc = io.tile([128, 2, C], f32, tag="xc")
                nc.sync.dma_start(
                    xc[:], x_d.ap()[t0:t0 + TN].rearrange(
                        "(i p) c -> p i c", p=128))
                for ti in range(2):
                    stats = st.tile([128, 3, 6], f32, tag="bnst")
                    for s in range(3):
                        nc.vector.bn_stats(
                            stats[:, s, :], xc[:, ti, 256 * s:256 * (s + 1)])
                    mv = st.tile([128, 2], f32, tag="mv")
                    nc.vector.bn_aggr(mv[:], stats[:])
                    sq = st.tile([128, 1], f32, tag="sq")
                    nc.scalar.activation(sq[:], mv[:, 1:2], AF.Sqrt,
                                         bias=eps_sb[:], scale=1.0)
                    rstd = st.tile([128, 1], f32, tag="rstd")
                    nc.vector.reciprocal(rstd[:], sq[:])
                    negmu = st.tile([128, 1], f32, tag="ngm")
                    nc.vector.tensor_scalar(
                        negmu[:], mv[:, 0:1], scalar1=rstd[:], scalar2=-1.0,
                        op0=AL.mult, op1=AL.mult)
                    nc.gpsimd.tensor_scalar(
                        xc[:, ti, :], xc[:, ti, :], scalar1=rstd[:],
                        scalar2=negmu[:], op0=AL.mult, op1=AL.add)

                # ---- transpose xn to channel-major [C, TN] ----
                xnT = pa.tile([128, KT, TN], f32r, tag="xnT")
                for ci in range(KT):
                    for ti in range(2):
                        pt = ps_t.tile([128, 128], f32, tag="ps")
                        nc.tensor.transpose(
                            pt[:], xc[:, ti, 128 * ci:128 * (ci + 1)],
                            ident[:])
                        nc.scalar.copy(
                            xnT[:, ci, 128 * ti:128 * (ti + 1)], pt[:])

                # ---- qk^T = Wqk^T @ xn^T, fused bias+relu+eps ----
                # mi 0..5 produce q (written per-head into qpad);
                # mi 6..11 produce k (written into kT).
                kT = pb.tile([128, KT, TN], f32r, tag="kT")
                for mi in range(12):
                    pq = ps_mm.tile([128, 512], f32, tag="ps")
                    for kt in range(KT):
                        nc.tensor.matmul(
                            pq[:, :TN],
                            wqk_sb[:, kt, 128 * mi:128 * (mi + 1)],
                            xnT[:, kt, :],
                            start=(kt == 0), stop=(kt == KT - 1))
                    if mi < 6:
                        for par in range(2):
                            h = 2 * mi + par
                            ro = par * 64
                            if qk_bias:
                                nc.vector.tensor_scalar(
                                    qpad[ro:ro + 64, h, :],
                                    pq[ro:ro + 64, :TN],
                                    scalar1=bqk_sb[ro:ro + 64, mi:mi + 1],
                                    scalar2=EPS_ATTN, op0=AL.add, op1=AL.max)
                            else:
                                nc.vector.tensor_scalar(
                                    qpad[ro:ro + 64, h, :],
                                    pq[ro:ro + 64, :TN],
                                    scalar1=EPS_ATTN, scalar2=EPS_ATTN,
                                    op0=AL.add, op1=AL.max)
                    else:
                        if qk_bias:
                            nc.vector.tensor_scalar(
                                kT[:, mi - 6, :], pq[:, :TN],
                                scalar1=bqk_sb[:, mi:mi + 1],
                                scalar2=EPS_ATTN, op0=AL.add, op1=AL.max)
                        else:
                            nc.vector.tensor_scalar(
                                kT[:, mi - 6, :], pq[:, :TN],
                                scalar1=EPS_ATTN, scalar2=EPS_ATTN,
                                op0=AL.add, op1=AL.max)

                # ---- v = xn @ Wv (token-major) ----
                v_sb = pb.tile([128, 2, C], f32r, tag="v")
                for ti in range(2):
                    for n0, nn in ((0, 512), (512, 256)):
                        pv = ps_mm.tile([128, 512], f32, tag="ps")
                        for kt in range(KT):
                            nc.tensor.matmul(
                                pv[:, :nn],
                                xnT[:, kt, 128 * ti:128 * (ti + 1)],
                                wv_sb[:, kt, n0:n0 + nn],
                                start=(kt == 0), stop=(kt == KT - 1))
                        if v_bias:
                            nc.vector.tensor_tensor(
                                v_sb[:, ti, n0:n0 + nn], pv[:, :nn],
                                bvbc_sb[:, n0:n0 + nn], op=AL.add)
                        else:
                            nc.scalar.copy(
                                v_sb[:, ti, n0:n0 + nn], pv[:, :nn])

                # ---- attention per head: S^T = K Q^T (scaled), o^T = V^T S^T
                # Software-pipelined: outT lags scores by LAG heads so PE is
                # not stalled on the elementwise piece-scale of the same head.
                oT = pa.tile([128, KT, TN], f32r, tag="oT")
                LAG = 3
                s_list = [None] * 12
                for step in range(12 + LAG):
                    if step < 12:
                        h = step
                        s_sb = sc.tile([128, 2, TN], f32r, tag="s")
                        s_list[h] = s_sb
                        for half in range(2):
                            pst = ps_att.tile([128, 256], f32, tag="ps")
                            nc.tensor.matmul(
                                pst[:],
                                kT[:, h // 2, 128 * half:128 * (half + 1)],
                                qpad[:, h, :],
                                start=True, stop=True)
                            if half == 0:
                                nc.vector.tensor_tensor(
                                    s_sb[:, half, :], pst[:],
                                    pbc_sb[:, half, :], op=AL.mult)
                            else:
                                nc.scalar.copy(s_sb[:, half, :], pst[:])
                                nc.gpsimd.tensor_tensor(
                                    s_sb[:, half, :], s_sb[:, half, :],
                                    pbc_sb[:, half, :], op=AL.mult)
                    if step >= LAG:
                        h = step - LAG
                        ro = (h % 2) * 64
                        s_sb = s_list[h]
                        po = ps_att.tile([128, 256], f32, tag="ps")
                        for kt in range(2):
                            nc.tensor.matmul(
                                po[:64, :],
                                v_sb[:, kt, 64 * h:64 * (h + 1)],
                                s_sb[:, kt, :],
                                start=(kt == 0), stop=(kt == 1))
                        nc.scalar.copy(oT[ro:ro + 64, h // 2, :], po[:64, :])

                # ---- y = out @ Wo + b_out (token-major) ----
                y_sb = io.tile([128, 2, C], f32, tag="y")
                for ti in range(2):
                    for n0, nn in ((0, 512), (512, 256)):
                        py = ps_mm.tile([128, 512], f32, tag="ps")
                        for kt in range(KT):
                            nc.tensor.matmul(
                                py[:, :nn],
                                oT[:, kt, 128 * ti:128 * (ti + 1)],
                                wo_sb[:, kt, n0:n0 + nn],
                                start=(kt == 0), stop=(kt == KT - 1))
                        if o_bias:
                            nc.vector.tensor_tensor(
                                y_sb[:, ti, n0:n0 + nn], py[:, :nn],
                                bobc_sb[:, n0:n0 + nn], op=AL.add)
                        else:
                            nc.scalar.copy(
                                y_sb[:, ti, n0:n0 + nn], py[:, :nn])
                nc.sync.dma_start(
                    y_d.ap()[t0:t0 + TN].rearrange("(i p) c -> p i c", p=128),
                    y_sb[:])

    nc.compile()
    return nc


def _get_program(qk_bias, v_bias, o_bias):
    key = (qk_bias, v_bias, o_bias)
    if key not in _CACHE:
        _CACHE[key] = _build_program(*key)
    return _CACHE[key]


def _round_f32r(a):
    """Round to the bf16-pair representable set (what fp32r matmuls use)."""
    import ml_dtypes
    hi = a.astype(ml_dtypes.bfloat16).astype(np.float32)
    lo = (a - hi).astype(ml_dtypes.bfloat16).astype(np.float32)
    return hi + lo


def kernel(x, ln_gamma, ln_beta, w_qkv, w_out, b_out, w_piece):
    global LAST_EXEC_NS, LAST_RESULTS
    from concourse.bass_utils import run_bass_kernel_spmd

    x = np.asarray(x, dtype=np.float32)
    ln_gamma = np.asarray(ln_gamma, dtype=np.float32)
    ln_beta = np.asarray(ln_beta, dtype=np.float32)
    w_qkv = np.asarray(w_qkv, dtype=np.float32)
    w_out = np.asarray(w_out, dtype=np.float32)
    b_out = np.asarray(b_out, dtype=np.float32)
    w_piece = np.asarray(w_piece, dtype=np.float32)

    # Host-side weight prep: fold gamma into qkv weights; beta becomes biases.
    wqk = _round_f32r(np.ascontiguousarray(ln_gamma[:, None] * w_qkv[:, :2 * C]))
    wv = _round_f32r(np.ascontiguousarray(ln_gamma[:, None] * w_qkv[:, 2 * C:]))
    wo = _round_f32r(np.ascontiguousarray(w_out))
    bqk = ln_beta @ w_qkv[:, :2 * C]
    bv = ln_beta @ w_qkv[:, 2 * C:]
    qk_bias = bool(np.any(bqk))
    v_bias = bool(np.any(bv))
    o_bias = bool(np.any(b_out))
    bqk_r = np.ascontiguousarray((bqk + EPS_ATTN).reshape(12, 128).T)
    bvbc = np.ascontiguousarray(np.broadcast_to(bv, (128, C)))
    bobc = np.ascontiguousarray(np.broadcast_to(b_out, (128, C)))
    # P^T broadcast tiles: pbc[p, half, f] = P[f//16, p//16 + 8*half]
    pk = np.kron(w_piece.T, np.ones((16, 16), dtype=np.float32))  # [256,256]
    pbc = np.ascontiguousarray(
        pk.reshape(2, 128, TN).transpose(1, 0, 2)).astype(np.float32)
    ident = np.eye(128, dtype=np.float32)

    shared = {
        "wqk": wqk, "wv": wv, "wo": wo, "bqk": bqk_r, "bvbc": bvbc,
        "bobc": bobc, "pbc": pbc, "ident": ident,
        "qzero": np.zeros((128, 12 * TN), dtype=np.float32),
    }
    xs = x.reshape(NCORES, TOK, C)
    in_maps = [dict(shared, x=np.ascontiguousarray(xs[i]))
               for i in range(NCORES)]

    nc = _get_program(qk_bias, v_bias, o_bias)
    trace = bool(os.environ.get("MHLA_TRACE"))
    res = run_bass_kernel_spmd(nc, in_maps, core_ids=list(range(NCORES)),
                               trace=trace)
    LAST_EXEC_NS = res.exec_time_ns
    LAST_RESULTS = res

    y = np.empty((NCORES, TOK, C), dtype=np.float32)
    for i in range(NCORES):
        y[i] = res.results[i]["y"]
    return y.reshape(B, N, W, C)


# revision 8
# speedup vs baseline: 1.2416x; 1.0092x over previous
"""Trainium2 Bass kernel for nn_MHLA_82695300317575.

Multi-head linear (relu-kernel) attention over 16-token windows with a
fixed 16x16 piece-mixing matrix, LayerNorm in, output projection out.

Strategy: pure data parallel over the batch dim (16 batches per core x 8
cores). Per (b,h) the window attention is computed in quadratic form:
  S^T = K Q^T  (256x256), scaled elementwise by kron(P, ones(16,16))^T,
  out^T = V^T S^T
which is mathematically identical to mixing the per-window kv matrices
with P and costs only large matmuls. All matmuls run as float32r
(full-rate fp32 PE mode); contraction dims are kept at the full 128
partitions (K<128 fp32r matmuls are ~4x slower), which is why q lives in
a zero-padded per-head tile. LayerNorm gamma is folded into the qkv
weights on the host; beta/b_out biases get dedicated program variants so
the common all-zero case skips the bias adds.
"""

import os
import sys

sys.path.insert(0, "/opt/trn_rl_repo")

import numpy as np

# Problem constants (hardcoded per harness contract)
B, N, W, C = 128, 16, 16, 768
H, D = 12, 64
NCORES = 8
BL = B // NCORES            # batches per core
TOK = BL * N * W            # tokens per core = 4096
TN = N * W                  # tokens per batch chunk = 256
EPS_ATTN = 1e-6
LN_EPS = 1e-5

LAST_EXEC_NS = None
LAST_RESULTS = None

_CACHE = {}


def _build_program(qk_bias, v_bias, o_bias):
    import concourse.tile as tile
    from concourse import bacc, mybir

    f32 = mybir.dt.float32
    f32r = mybir.dt.float32r
    AL = mybir.AluOpType
    AF = mybir.ActivationFunctionType

    nc = bacc.Bacc("TRN2", target_bir_lowering=False, debug=False,
                   num_devices=NCORES)

    x_d = nc.dram_tensor("x", [TOK, C], f32, kind="ExternalInput")
    wqk_d = nc.dram_tensor("wqk", [C, 2 * C], f32r, kind="ExternalInput")
    wv_d = nc.dram_tensor("wv", [C, C], f32r, kind="ExternalInput")
    wo_d = nc.dram_tensor("wo", [C, C], f32r, kind="ExternalInput")
    bqk_d = nc.dram_tensor("bqk", [128, 12], f32, kind="ExternalInput")
    bvbc_d = nc.dram_tensor("bvbc", [128, C], f32, kind="ExternalInput")
    bobc_d = nc.dram_tensor("bobc", [128, C], f32, kind="ExternalInput")
    pbc_d = nc.dram_tensor("pbc", [128, 2, TN], f32, kind="ExternalInput")
    ident_d = nc.dram_tensor("ident", [128, 128], f32, kind="ExternalInput")
    qz_d = nc.dram_tensor("qzero", [128, 12 * TN], f32r, kind="ExternalInput")
    y_d = nc.dram_tensor("y", [TOK, C], f32, kind="ExternalOutput")

    KT = C // 128            # 6 k-tiles over channels

    with tile.TileContext(nc) as tc:
        with (
            tc.tile_pool(name="singles", bufs=1) as singles,
            tc.tile_pool(name="io", bufs=2) as io,
            tc.tile_pool(name="pa", bufs=2) as pa,
            tc.tile_pool(name="pb", bufs=1) as pb,
            tc.tile_pool(name="sc", bufs=7) as sc,
            tc.tile_pool(name="st", bufs=4) as st,
            tc.tile_pool(name="ps_t", bufs=2, space="PSUM") as ps_t,
            tc.tile_pool(name="ps_mm", bufs=3, space="PSUM") as ps_mm,
            tc.tile_pool(name="ps_att", bufs=3, space="PSUM") as ps_att,
        ):
            wqk_sb = singles.tile([128, KT, 2 * C], f32r)
            nc.gpsimd.dma_start(wqk_sb[:], wqk_d.ap().rearrange(
                "(kt p) m -> p kt m", p=128))
            wv_sb = singles.tile([128, KT, C], f32r)
            nc.gpsimd.dma_start(wv_sb[:], wv_d.ap().rearrange(
                "(kt p) m -> p kt m", p=128))
            wo_sb = singles.tile([128, KT, C], f32r)
            nc.gpsimd.dma_start(wo_sb[:], wo_d.ap().rearrange(
                "(kt p) m -> p kt m", p=128))
            if qk_bias:
                bqk_sb = singles.tile([128, 12], f32)
                nc.gpsimd.dma_start(bqk_sb[:], bqk_d.ap())
            if v_bias:
                bvbc_sb = singles.tile([128, C], f32)
                nc.gpsimd.dma_start(bvbc_sb[:], bvbc_d.ap())
            if o_bias:
                bobc_sb = singles.tile([128, C], f32)
                nc.gpsimd.dma_start(bobc_sb[:], bobc_d.ap())
            pbc_sb = singles.tile([128, 2, TN], f32)
            nc.gpsimd.dma_start(pbc_sb[:], pbc_d.ap())
            ident = singles.tile([128, 128], f32)
            nc.sync.dma_start(ident[:], ident_d.ap())
            eps_sb = singles.tile([128, 1], f32)
            nc.vector.memset(eps_sb[:], LN_EPS)

            # Persistent zero-padded q tiles (double buffered by chunk
            # parity). Per head h, rows (h%2)*64..+64 hold relu(q_h)+eps;
            # the other 64 rows stay zero so the scores matmul can run with
            # the full K=128 contraction against a k tile whose complementary
            # rows hold the sibling head (junk x 0 = 0).
            qpads = []
            for i in range(2):
                qp = singles.tile([128, 12, TN], f32r, tag=f"qpad{i}")
                nc.gpsimd.dma_start(
                    qp[:], qz_d.ap().rearrange("p (h t) -> p h t", h=12))
                qpads.append(qp)

            for chunk in range(BL):
                t0 = chunk * TN
                qpad = qpads[chunk % 2]

                # ---- load + LayerNorm (token-major, in place) ----
                xc = io.tile([128, 2, C], f32, tag="xc")
                nc.sync.dma_start(
                    xc[:], x_d.ap()[t0:t0 + TN].rearrange(
                        "(i p) c -> p i c", p=128))
                for ti in range(2):
                    stats = st.tile([128, 3, 6], f32, tag="bnst")
                    for s in range(3):
                        nc.vector.bn_stats(
                            stats[:, s, :], xc[:, ti, 256 * s:256 * (s + 1)])
                    mv = st.tile([128, 2], f32, tag="mv")
                    nc.vector.bn_aggr(mv[:], stats[:])
                    sq = st.tile([128, 1], f32, tag="sq")
                    nc.scalar.activation(sq[:], mv[:, 1:2], AF.Sqrt,
                                         bias=eps_sb[:], scale=1.0)
                    rstd = st.tile([128, 1], f32, tag="rstd")
                    nc.vector.reciprocal(rstd[:], sq[:])
                    negmu = st.tile([128, 1], f32, tag="ngm")
                    nc.vector.tensor_scalar(
                        negmu[:], mv[:, 0:1], scalar1=rstd[:], scalar2=-1.0,
                        op0=AL.mult, op1=AL.mult)
                    nc.gpsimd.tensor_scalar(
                        xc[:, ti, :], xc[:, ti, :], scalar1=rstd[:],
                        scalar2=negmu[:], op0=AL.mult, op1=AL.add)

                # ---- transpose xn to channel-major [C, TN] ----
                xnT = pa.tile([128, KT, TN], f32r, tag="xnT")
                for ci in range(KT):
                    for ti in range(2):
                        pt = ps_t.tile([128, 128], f32, tag="ps")
                        nc.tensor.transpose(
                            pt[:], xc[:, ti, 128 * ci:128 * (ci + 1)],
                            ident[:])
                        nc.scalar.copy(
                            xnT[:, ci, 128 * ti:128 * (ti + 1)], pt[:])

                # ---- qk^T = Wqk^T @ xn^T, fused bias+relu+eps ----
                # mi 0..5 produce q (written per-head into qpad);
                # mi 6..11 produce k (written into kT).
                kT = pb.tile([128, KT, TN], f32r, tag="kT")
                for mi in range(12):
                    pq = ps_mm.tile([128, 512], f32, tag="ps")
                    for kt in range(KT):
                        nc.tensor.matmul(
                            pq[:, :TN],
                            wqk_sb[:, kt, 128 * mi:128 * (mi + 1)],
                            xnT[:, kt, :],
                            start=(kt == 0), stop=(kt == KT - 1))
                    if mi < 6:
                        for par in range(2):
                            h = 2 * mi + par
                            ro = par * 64
                            if qk_bias:
                                nc.vector.tensor_scalar(
                                    qpad[ro:ro + 64, h, :],
                                    pq[ro:ro + 64, :TN],
                                    scalar1=bqk_sb[ro:ro + 64, mi:mi + 1],
                                    scalar2=EPS_ATTN, op0=AL.add, op1=AL.max)
                            else:
                                nc.vector.tensor_scalar(
                                    qpad[ro:ro + 64, h, :],
                                    pq[ro:ro + 64, :TN],
                                    scalar1=EPS_ATTN, scalar2=EPS_ATTN,
                                    op0=AL.add, op1=AL.max)
                    else:
                        if qk_bias:
                            nc.vector.tensor_scalar(
                                kT[:, mi - 6, :], pq[:, :TN],
                                scalar1=bqk_sb[:, mi:mi + 1],
                                scalar2=EPS_ATTN, op0=AL.add, op1=AL.max)
                        else:
                            nc.vector.tensor_scalar(
                                kT[:, mi - 6, :], pq[:, :TN],
                                scalar1=EPS_ATTN, scalar2=EPS_ATTN,
                                op0=AL.add, op1=AL.max)

                # ---- v = xn @ Wv (token-major) ----
                v_sb = pb.tile([128, 2, C], f32r, tag="v")
                for ti in range(2):
                    for n0, nn in ((0, 512), (512, 256)):
                        pv = ps_mm.tile([128, 512], f32, tag="ps")
                        for kt in range(KT):
                            nc.tensor.matmul(
                                pv[:, :nn],
                                xnT[:, kt, 128 * ti:128 * (ti + 1)],
                                wv_sb[:, kt, n0:n0 + nn],
                                start=(kt == 0), stop=(kt == KT - 1))
                        if v_bias:
                            nc.vector.tensor_tensor(
                                v_sb[:, ti, n0:n0 + nn], pv[:, :nn],
                                bvbc_sb[:, n0:n0 + nn], op=AL.add)
                        else:
                            nc.scalar.copy(
                                v_sb[:, ti, n0:n0 + nn], pv[:, :nn])

                # ---- attention per head: S^T = K Q^T (scaled), o^T = V^T S^T
                # Software-pipelined: outT lags scores by LAG heads so PE is
                # not stalled on the elementwise piece-scale of the same head.
                oT = pa.tile([128, KT, TN], f32r, tag="oT")
                LAG = 5
                s_list = [None] * 12
                for step in range(12 + LAG):
                    if step < 12:
                        h = step
                        s_sb = sc.tile([128, 2, TN], f32r, tag="s")
                        s_list[h] = s_sb
                        for half in range(2):
                            pst = ps_att.tile([128, 256], f32, tag="ps")
                            nc.tensor.matmul(
                                pst[:],
                                kT[:, h // 2, 128 * half:128 * (half + 1)],
                                qpad[:, h, :],
                                start=True, stop=True)
                            if half == 0:
                                nc.vector.tensor_tensor(
                                    s_sb[:, half, :], pst[:],
                                    pbc_sb[:, half, :], op=AL.mult)
                            else:
                                nc.scalar.copy(s_sb[:, half, :], pst[:])
                                nc.gpsimd.tensor_tensor(
                                    s_sb[:, half, :], s_sb[:, half, :],
                                    pbc_sb[:, half, :], op=AL.mult)
                    if step >= LAG:
                        h = step - LAG
                        ro = (h % 2) * 64
                        s_sb = s_list[h]
                        po = ps_att.tile([128, 256], f32, tag="ps")
                        for kt in range(2):
                            nc.tensor.matmul(
                                po[:64, :],
                                v_sb[:, kt, 64 * h:64 * (h + 1)],
                                s_sb[:, kt, :],
                                start=(kt == 0), stop=(kt == 1))
                        nc.scalar.copy(oT[ro:ro + 64, h // 2, :], po[:64, :])

                # ---- y = out @ Wo + b_out (token-major) ----
                y_sb = io.tile([128, 2, C], f32, tag="y")
                for ti in range(2):
                    for n0, nn in ((0, 512), (512, 256)):
                        py = ps_mm.tile([128, 512], f32, tag="ps")
                        for kt in range(KT):
                            nc.tensor.matmul(
                                py[:, :nn],
                                oT[:, kt, 128 * ti:128 * (ti + 1)],
                                wo_sb[:, kt, n0:n0 + nn],
                                start=(kt == 0), stop=(kt == KT - 1))
                        if o_bias:
                            nc.vector.tensor_tensor(
                                y_sb[:, ti, n0:n0 + nn], py[:, :nn],
                                bobc_sb[:, n0:n0 + nn], op=AL.add)
                        else:
                            nc.scalar.copy(
                                y_sb[:, ti, n0:n0 + nn], py[:, :nn])
                nc.sync.dma_start(
                    y_d.ap()[t0:t0 + TN].rearrange("(i p) c -> p i c", p=128),
                    y_sb[:])

    nc.compile()
    return nc


def _get_program(qk_bias, v_bias, o_bias):
    key = (qk_bias, v_bias, o_bias)
    if key not in _CACHE:
        _CACHE[key] = _build_program(*key)
    return _CACHE[key]


def _round_f32r(a):
    """Round to the bf16-pair representable set (what fp32r matmuls use)."""
    import ml_dtypes
    hi = a.astype(ml_dtypes.bfloat16).astype(np.float32)
    lo = (a - hi).astype(ml_dtypes.bfloat16).astype(np.float32)
    return hi + lo


def kernel(x, ln_gamma, ln_beta, w_qkv, w_out, b_out, w_piece):
    global LAST_EXEC_NS, LAST_RESULTS
    from concourse.bass_utils import run_bass_kernel_spmd

    x = np.asarray(x, dtype=np.float32)
    ln_gamma = np.asarray(ln_gamma, dtype=np.float32)
    ln_beta = np.asarray(ln_beta, dtype=np.float32)
    w_qkv = np.asarray(w_qkv, dtype=np.float32)
    w_out = np.asarray(w_out, dtype=np.float32)
    b_out = np.asarray(b_out, dtype=np.float32)
    w_piece = np.asarray(w_piece, dtype=np.float32)

    # Host-side weight prep: fold gamma into qkv weights; beta becomes biases.
    wqk = _round_f32r(np.ascontiguousarray(ln_gamma[:, None] * w_qkv[:, :2 * C]))
    wv = _round_f32r(np.ascontiguousarray(ln_gamma[:, None] * w_qkv[:, 2 * C:]))
    wo = _round_f32r(np.ascontiguousarray(w_out))
    bqk = ln_beta @ w_qkv[:, :2 * C]
    bv = ln_beta @ w_qkv[:, 2 * C:]
    qk_bias = bool(np.any(bqk))
    v_bias = bool(np.any(bv))
    o_bias = bool(np.any(b_out))
    bqk_r = np.ascontiguousarray((bqk + EPS_ATTN).reshape(12, 128).T)
    bvbc = np.ascontiguousarray(np.broadcast_to(bv, (128, C)))
    bobc = np.ascontiguousarray(np.broadcast_to(b_out, (128, C)))
    # P^T broadcast tiles: pbc[p, half, f] = P[f//16, p//16 + 8*half]
    pk = np.kron(w_piece.T, np.ones((16, 16), dtype=np.float32))  # [256,256]
    pbc = np.ascontiguousarray(
        pk.reshape(2, 128, TN).transpose(1, 0, 2)).astype(np.float32)
    ident = np.eye(128, dtype=np.float32)

    shared = {
        "wqk": wqk, "wv": wv, "wo": wo, "bqk": bqk_r, "bvbc": bvbc,
        "bobc": bobc, "pbc": pbc, "ident": ident,
        "qzero": np.zeros((128, 12 * TN), dtype=np.float32),
    }
    xs = x.reshape(NCORES, TOK, C)
    in_maps = [dict(shared, x=np.ascontiguousarray(xs[i]))
               for i in range(NCORES)]

    nc = _get_program(qk_bias, v_bias, o_bias)
    trace = bool(os.environ.get("MHLA_TRACE"))
    res = run_bass_kernel_spmd(nc, in_maps, core_ids=list(range(NCORES)),
                               trace=trace)
    LAST_EXEC_NS = res.exec_time_ns
    LAST_RESULTS = res

    y = np.empty((NCORES, TOK, C), dtype=np.float32)
    for i in range(NCORES):
        y[i] = res.results[i]["y"]
    return y.reshape(B, N, W, C)
